# revision 8
# baseline (speedup 1.0000x reference)
"""Trainium2 Bass kernel for nn_Attention_28862180229709.

Head-sharded (2 heads/core x 8 cores) fused attention:
  LayerNorm -> Q/KV projections -> interleaved RoPE -> per-head bilinear K
  transform -> softmax(QK^T)V -> output projection (row-parallel Wo),
  host-side sum of the 8 partial outputs.

Layout strategy (per core):
  - xn is transposed on-chip (PE transpose) to xnT [c, n] so all projections
    contract c on the partition axis.
  - q/k are produced transposed ([d, n]) with the head-dim permuted to
    [evens, odds] per head so RoPE's interleaved pair-swap becomes a
    contiguous 32-partition block swap (plain strided DVE ops).
  - scores are computed transposed (simT [keys, qrows]) so softmax
    normalization folds into the output side and attn @ V needs no
    transposes at all; row-sums come from an appended ones-column on V.
  - all matmuls run in bf16 (fp32 PSUM accumulation); LayerNorm, RoPE and
    softmax run in fp32 on DVE/ACT.
"""

import os
import sys

for _p in ("/opt/trn_rl_repo", "/root/.axon_site/_ro/trn_rl_repo"):
    if os.path.isdir(_p) and _p not in sys.path:
        sys.path.insert(0, _p)

from contextlib import ExitStack

import ml_dtypes
import numpy as np

import concourse.bacc as bacc
import concourse.tile as tile
from concourse import mybir
from concourse.bass_utils import run_bass_kernel_spmd

P = 128
DIM = 1024
HEADS = 16
DHEAD = 64
INNER = HEADS * DHEAD
NCORES = 8
HPC = HEADS // NCORES  # heads per core (2)
CB = DIM // P  # contraction chunks (8)
IB = 512  # i-block (psum bank) width
ROPE_BASE = 10000.0
LN_EPS = 1e-5

F32 = mybir.dt.float32
BF16 = mybir.dt.bfloat16
AF = mybir.ActivationFunctionType
ALU = mybir.AluOpType

# per-head dim permutation: evens then odds (so RoPE pairs are 32 apart)
_PERM = np.concatenate([np.arange(0, DHEAD, 2), np.arange(1, DHEAD, 2)])


def _build_nc(N, debug_taps=False):
    """Build the SPMD Bass program for sequence length N (tokens)."""
    NT = N // P  # token tiles
    NIB = N // IB  # i-blocks
    assert N % IB == 0

    nc = bacc.Bacc("TRN2", target_bir_lowering=False, debug=False)

    x_d = nc.dram_tensor("x", (N, DIM), F32, kind="ExternalInput")
    wq_d = nc.dram_tensor("wq", (CB, P, P), BF16, kind="ExternalInput")
    wk_d = nc.dram_tensor("wk", (CB, P, P), BF16, kind="ExternalInput")
    wv_d = nc.dram_tensor("wv", (CB, P, P), BF16, kind="ExternalInput")
    wb_d = nc.dram_tensor("wb", (P, P), BF16, kind="ExternalInput")
    wo_d = nc.dram_tensor("wo", (P, DIM), BF16, kind="ExternalInput")
    cos_d = nc.dram_tensor("cosT", (P, N), F32, kind="ExternalInput")
    sin_d = nc.dram_tensor("sinT", (P, N), F32, kind="ExternalInput")
    id_d = nc.dram_tensor("ident", (P, P), BF16, kind="ExternalInput")
    out_d = nc.dram_tensor("out", (N, DIM), F32, kind="ExternalOutput")
    if debug_taps:
        dbg = {
            "dbg_xnT": nc.dram_tensor("dbg_xnT", (P, CB, N), BF16, kind="ExternalOutput"),
            "dbg_q": nc.dram_tensor("dbg_q", (P, N), BF16, kind="ExternalOutput"),
            "dbg_k": nc.dram_tensor("dbg_k", (P, N), BF16, kind="ExternalOutput"),
            "dbg_kt": nc.dram_tensor("dbg_kt", (P, N), BF16, kind="ExternalOutput"),
            "dbg_e0": nc.dram_tensor("dbg_e0", (2, P, N), BF16, kind="ExternalOutput"),
            "dbg_r": nc.dram_tensor("dbg_r", (2, 1, N), F32, kind="ExternalOutput"),
            "dbg_osc": nc.dram_tensor("dbg_osc", (P, N), BF16, kind="ExternalOutput"),
            "dbg_v": nc.dram_tensor("dbg_v", (P, 16, 130), BF16, kind="ExternalOutput"),
        }

    with tile.TileContext(nc) as tc, ExitStack() as ctx:
        const = ctx.enter_context(tc.tile_pool(name="const", bufs=1))
        big = ctx.enter_context(tc.tile_pool(name="big", bufs=1))

        wq_sb = const.tile([P, CB, P], BF16)
        wk_sb = const.tile([P, CB, P], BF16)
        wv_sb = const.tile([P, CB, P], BF16)
        wb_sb = const.tile([P, P], BF16)
        wo_sb = const.tile([P, DIM], BF16)
        cos_sb = const.tile([P, N], F32)
        sin_sb = const.tile([P, N], F32)
        id_sb = const.tile([P, P], BF16)
        eps_sb = const.tile([P, 1], F32)
        zero_sb = const.tile([P, 1], F32)
        nc.vector.memset(eps_sb[:], LN_EPS)
        nc.vector.memset(zero_sb[:], 0.0)
        nc.sync.dma_start(wq_sb[:], wq_d[:].rearrange("a p m -> p a m"))
        nc.sync.dma_start(wk_sb[:], wk_d[:].rearrange("a p m -> p a m"))
        nc.sync.dma_start(wv_sb[:], wv_d[:].rearrange("a p m -> p a m"))
        nc.sync.dma_start(wb_sb[:], wb_d[:])
        nc.sync.dma_start(wo_sb[:], wo_d[:])
        nc.sync.dma_start(cos_sb[:], cos_d[:])
        nc.sync.dma_start(sin_sb[:], sin_d[:])
        nc.sync.dma_start(id_sb[:], id_d[:])

        # long-lived activations
        xnT = big.tile([P, CB, N], BF16)  # xn transposed, c on partitions
        q_rope = big.tile([P, N], BF16)
        ktT = big.tile([P, N], BF16)
        v_sb = big.tile([P, NT, 2 * (DHEAD + 1)], BF16)  # [keys, tile, head|ones]
        outT_sc = big.tile([P, N], BF16)  # scaled attn out, d on partitions

        nc.gpsimd.memset(v_sb[:], 1.0)

        # ---- Phase A: load x, LayerNorm, transpose to xnT ----
        with ExitStack() as actx:
            xp = actx.enter_context(tc.tile_pool(name="xp", bufs=3))
            sp = actx.enter_context(tc.tile_pool(name="sp", bufs=4))
            xnp = actx.enter_context(tc.tile_pool(name="xnp", bufs=3))
            tp = actx.enter_context(tc.tile_pool(name="tp", bufs=1, space="PSUM"))

            n_group = 4  # token tiles per transpose-psum batch
            for tg in range(NT // n_group):
                ps_t = [tp.tile([P, n_group * P], BF16, tag=f"t{cb}", name=f"ps_t{cb}") for cb in range(CB)]
                for ti in range(n_group):
                    t = tg * n_group + ti
                    xt = xp.tile([P, DIM], F32, tag="x")
                    nc.sync.dma_start(xt[:], x_d[t * P : (t + 1) * P, :])
                    st = sp.tile([P, 2, 6], F32, tag="st")
                    nc.vector.bn_stats(st[:, 0, :], xt[:, 0:512])
                    nc.vector.bn_stats(st[:, 1, :], xt[:, 512:1024])
                    mv = sp.tile([P, 2], F32, tag="mv")
                    nc.vector.bn_aggr(mv[:], st[:])
                    rstd = sp.tile([P, 1], F32, tag="rstd")
                    nc.scalar.activation(rstd[:], mv[:, 1:2], AF.Sqrt, bias=eps_sb[:])
                    nc.vector.reciprocal(rstd[:], rstd[:])
                    xn = xnp.tile([P, DIM], BF16, tag="xn")
                    nc.vector.tensor_scalar(
                        xn[:], xt[:], mv[:, 0:1], rstd[:], ALU.subtract, ALU.mult
                    )
                    for cb in range(CB):
                        nc.tensor.transpose(
                            ps_t[cb][:, ti * P : (ti + 1) * P],
                            xn[:, cb * P : (cb + 1) * P],
                            id_sb[:],
                        )
                for cb in range(CB):
                    dst = xnT[:, cb, tg * n_group * P : (tg + 1) * n_group * P]
                    if cb % 2 == 0:
                        nc.vector.tensor_copy(dst, ps_t[cb][:])
                    else:
                        nc.scalar.copy(dst, ps_t[cb][:])

        # ---- Phase A2: projections + rope + bilinear + v ----
        with ExitStack() as actx:
            qkps = actx.enter_context(tc.tile_pool(name="qkps", bufs=1, space="PSUM"))
            vps = actx.enter_context(tc.tile_pool(name="vps", bufs=2, space="PSUM"))
            rtmp = actx.enter_context(tc.tile_pool(name="rtmp", bufs=2))

            k_rope = rtmp.tile([P, N], BF16, tag="krope")

            for w_sb, dst in ((wq_sb, q_rope), (wk_sb, k_rope)):
                ps_q = qkps.tile([P, N], F32, tag="qk")
                for ib in range(NIB):
                    sl = slice(ib * IB, (ib + 1) * IB)
                    for cb in range(CB):
                        nc.tensor.matmul(
                            ps_q[:, sl],
                            w_sb[:, cb, :],
                            xnT[:, cb, sl],
                            start=(cb == 0),
                            stop=(cb == CB - 1),
                        )
                tcos = rtmp.tile([P, N], F32, tag="tcos")
                nc.vector.tensor_mul(tcos[:], ps_q[:], cos_sb[:])
                tsin = rtmp.tile([P, N], F32, tag="tsin")
                for h in range(HPC):
                    for half in range(2):
                        o0 = h * DHEAD + half * 32
                        i0 = h * DHEAD + (1 - half) * 32
                        nc.vector.tensor_mul(
                            tsin[o0 : o0 + 32, :],
                            ps_q[i0 : i0 + 32, :],
                            sin_sb[o0 : o0 + 32, :],
                        )
                nc.vector.tensor_add(dst[:], tcos[:], tsin[:])
            if debug_taps:
                nc.sync.dma_start(dbg["dbg_k"][:], k_rope[:])

            # bilinear: ktT = Wb_block^T-contract over k-dims (block diagonal)
            ps_kt = qkps.tile([P, N], F32, tag="qk")
            for ib in range(NIB):
                sl = slice(ib * IB, (ib + 1) * IB)
                nc.tensor.matmul(
                    ps_kt[:, sl], wb_sb[:], k_rope[:, sl], start=True, stop=True
                )
                nc.vector.tensor_copy(ktT[:, sl], ps_kt[:, sl])

            # v in natural layout [keys, d] with ones columns for row-sums
            VW = DHEAD + 1
            for t in range(NT):
                ps_v = vps.tile([P, P], F32, tag="v")
                for cb in range(CB):
                    nc.tensor.matmul(
                        ps_v[:],
                        xnT[:, cb, t * P : (t + 1) * P],
                        wv_sb[:, cb, :],
                        start=(cb == 0),
                        stop=(cb == CB - 1),
                    )
                nc.vector.tensor_copy(v_sb[:, t, 0:DHEAD], ps_v[:, 0:DHEAD])
                nc.scalar.copy(v_sb[:, t, VW : VW + DHEAD], ps_v[:, DHEAD : 2 * DHEAD])

        # ---- Phase B: attention per head ----
        with ExitStack() as actx:
            sps = actx.enter_context(tc.tile_pool(name="sps", bufs=1, space="PSUM"))
            avps = actx.enter_context(tc.tile_pool(name="avps", bufs=1, space="PSUM"))
            ep = actx.enter_context(tc.tile_pool(name="ep", bufs=1))
            rp = actx.enter_context(tc.tile_pool(name="rp", bufs=2))

            VW = DHEAD + 1
            for h in range(HPC):
                hl = slice(h * DHEAD, (h + 1) * DHEAD)
                expT = []
                for j in range(NT):
                    ps_s = sps.tile([P, N], F32, tag="sim")
                    for ib in range(NIB):
                        sl = slice(ib * IB, (ib + 1) * IB)
                        nc.tensor.matmul(
                            ps_s[:, sl],
                            ktT[hl, j * P : (j + 1) * P],
                            q_rope[hl, sl],
                            start=True,
                            stop=True,
                        )
                    e_j = ep.tile([P, N], BF16, tag=f"e{j}")
                    nc.scalar.activation(e_j[:], ps_s[:], AF.Exp, bias=zero_sb[:])
                    if debug_taps and j == 0:
                        nc.sync.dma_start(dbg["dbg_e0"][h], e_j[:])
                    expT.append(e_j)
                ps_av = avps.tile([DHEAD + 1, N], F32, tag="av")
                for j in range(NT):
                    for ib in range(NIB):
                        sl = slice(ib * IB, (ib + 1) * IB)
                        nc.tensor.matmul(
                            ps_av[:, sl],
                            v_sb[:, j, h * VW : (h + 1) * VW],
                            expT[j][:, sl],
                            start=(j == 0),
                            stop=(j == NT - 1),
                        )
                rs_h = rp.tile([1, N], F32, tag="rs")
                nc.scalar.copy(rs_h[:], ps_av[DHEAD : DHEAD + 1, :])
                r_h = rp.tile([1, N], F32, tag="r")
                nc.vector.reciprocal_approx_fast(r_h[:], rs_h[:])
                if debug_taps:
                    nc.sync.dma_start(dbg["dbg_r"][h], r_h[:])
                rb_h = rp.tile([P, N], F32, tag="rb")
                nc.gpsimd.partition_broadcast(rb_h[:], r_h[:])
                nc.vector.tensor_mul(
                    outT_sc[hl, :], ps_av[0:DHEAD, :], rb_h[hl, :]
                )

        if debug_taps:
            nc.sync.dma_start(dbg["dbg_xnT"][:], xnT[:])
            nc.sync.dma_start(dbg["dbg_q"][:], q_rope[:])
            nc.sync.dma_start(dbg["dbg_kt"][:], ktT[:])
            nc.sync.dma_start(dbg["dbg_osc"][:], outT_sc[:])
            nc.sync.dma_start(dbg["dbg_v"][:], v_sb[:])

        # ---- Phase C: output projection ----
        with ExitStack() as actx:
            wops = actx.enter_context(tc.tile_pool(name="wops", bufs=2, space="PSUM"))
            op = actx.enter_context(tc.tile_pool(name="op", bufs=3))
            for t in range(NT):
                ps_o = wops.tile([P, DIM], F32, tag="o")
                for cc in range(2):
                    nc.tensor.matmul(
                        ps_o[:, cc * IB : (cc + 1) * IB],
                        outT_sc[:, t * P : (t + 1) * P],
                        wo_sb[:, cc * IB : (cc + 1) * IB],
                        start=True,
                        stop=True,
                    )
                o_sb = op.tile([P, DIM], F32, tag="osb")
                nc.vector.tensor_copy(o_sb[:, 0:IB], ps_o[:, 0:IB])
                nc.scalar.copy(o_sb[:, IB:DIM], ps_o[:, IB:DIM])
                nc.sync.dma_start(out_d[t * P : (t + 1) * P, :], o_sb[:])

    nc.compile()
    return nc


def _rope_tables(N):
    theta = 1.0 / (ROPE_BASE ** (np.arange(0, DHEAD, 2, dtype=np.float64) / DHEAD))
    pos = np.arange(N, dtype=np.float64)
    freqs = pos[:, None] * theta[None, :]  # [N, 32]
    emb = np.concatenate([freqs, freqs], axis=-1)  # [N, 64]
    cos, sin = np.cos(emb), np.sin(emb)  # [N, 64]
    # permuted-transposed tables for one head, rows [evens(32) | odds(32)]
    cosT = np.empty((DHEAD, N))
    sinT = np.empty((DHEAD, N))
    for r in range(32):
        cosT[r] = cos[:, 2 * r]
        cosT[32 + r] = cos[:, 2 * r + 1]
        sinT[r] = -sin[:, 2 * r]
        sinT[32 + r] = sin[:, 2 * r + 1]
    cosT2 = np.concatenate([cosT, cosT], axis=0).astype(np.float32)  # [128, N]
    sinT2 = np.concatenate([sinT, sinT], axis=0).astype(np.float32)
    return np.ascontiguousarray(cosT2), np.ascontiguousarray(sinT2)


def _prep_inputs(x, gamma, Wq, Wkv, W_bilinear, Wo):
    """Slice/permute weights per core; returns list of 8 input dicts."""
    b, N, _ = x.shape
    x2d = np.ascontiguousarray(x.reshape(N, DIM)).astype(np.float32)
    cosT, sinT = _rope_tables(N)
    ident = np.eye(P, dtype=ml_dtypes.bfloat16)

    g = gamma.astype(np.float64)
    Wqg = g[:, None] * Wq.astype(np.float64) * (DHEAD**-0.5)
    Wkg = g[:, None] * Wkv[:, :INNER].astype(np.float64)
    Wvg = g[:, None] * Wkv[:, INNER:].astype(np.float64)

    in_maps = []
    for c in range(NCORES):
        heads = [HPC * c + i for i in range(HPC)]
        qcols = np.concatenate([h * DHEAD + _PERM for h in heads])
        vcols = np.concatenate(
            [np.arange(h * DHEAD, (h + 1) * DHEAD) for h in heads]
        )
        wq_c = Wqg[:, qcols].astype(ml_dtypes.bfloat16).reshape(CB, P, P)
        wk_c = Wkg[:, qcols].astype(ml_dtypes.bfloat16).reshape(CB, P, P)
        wv_c = Wvg[:, vcols].astype(ml_dtypes.bfloat16).reshape(CB, P, P)
        wb_c = np.zeros((P, P), dtype=np.float64)
        for i, h in enumerate(heads):
            wb_h = W_bilinear[h].astype(np.float64)[_PERM][:, _PERM]
            wb_c[
                i * DHEAD : (i + 1) * DHEAD, i * DHEAD : (i + 1) * DHEAD
            ] = wb_h
        wo_c = Wo[vcols, :].astype(ml_dtypes.bfloat16)
        in_maps.append(
            {
                "x": x2d,
                "wq": np.ascontiguousarray(wq_c),
                "wk": np.ascontiguousarray(wk_c),
                "wv": np.ascontiguousarray(wv_c),
                "wb": np.ascontiguousarray(wb_c.astype(ml_dtypes.bfloat16)),
                "wo": np.ascontiguousarray(wo_c),
                "cosT": cosT,
                "sinT": sinT,
                "ident": ident,
            }
        )
    return in_maps


_NC_CACHE = {}


def _get_nc(N):
    if N not in _NC_CACHE:
        _NC_CACHE[N] = _build_nc(N)
    return _NC_CACHE[N]


def kernel(x, gamma, Wq, Wkv, W_bilinear, Wo, _trace=False, _trace_kwargs=None):
    b, N, dim = x.shape
    assert b == 1 and dim == DIM
    nc = _get_nc(N)
    in_maps = _prep_inputs(x, gamma, Wq, Wkv, W_bilinear, Wo)
    kw = {}
    if _trace:
        kw = {"trace": True, **(_trace_kwargs or {})}
    res = run_bass_kernel_spmd(nc, in_maps, core_ids=list(range(NCORES)), **kw)
    acc = np.zeros((N, DIM), dtype=np.float64)
    for c in range(NCORES):
        acc += res.results[c]["out"].astype(np.float64)
    out = acc.astype(np.float32).reshape(1, N, DIM)
    if _trace:
        return out, res
    return out


# revision 11
# speedup vs baseline: 1.0318x; 1.0318x over previous
"""Trainium2 Bass kernel for nn_Attention_28862180229709.

Head-sharded (2 heads/core x 8 cores) fused attention:
  LayerNorm -> Q/KV projections -> interleaved RoPE -> per-head bilinear K
  transform -> softmax(QK^T)V -> output projection (row-parallel Wo),
  host-side sum of the 8 partial outputs.

Layout strategy (per core):
  - xn is transposed on-chip (PE transpose) to xnT [c, n] so all projections
    contract c on the partition axis.
  - q/k are produced transposed ([d, n]) with the head dims permuted to a
    global [h0-evens | h1-evens | h0-odds | h1-odds] row order so RoPE's
    interleaved pair-swap becomes a single 64-partition block swap
    (partner = row ^ 64) done with two strided DVE multiplies.
  - the per-head bilinear K transform is emitted as two scattered
    block-diagonal weight matmuls producing zero-padded ktT_h tensors, so
    the QK^T matmuls contract the full K=128 partition range (K=64 matmuls
    never warm the PE clock gate - measured 427ns vs 215ns at N=512).
  - scores are computed transposed (simT [keys, qrows]) so softmax
    normalization folds into the output side and attn @ V needs no
    transposes; row-sums come from an appended ones-column on V.
  - all matmuls run in bf16 (fp32 PSUM accumulation); LayerNorm, RoPE and
    softmax run in fp32 on DVE/ACT.
"""

import os
import sys

for _p in ("/opt/trn_rl_repo", "/root/.axon_site/_ro/trn_rl_repo"):
    if os.path.isdir(_p) and _p not in sys.path:
        sys.path.insert(0, _p)

from contextlib import ExitStack

import ml_dtypes
import numpy as np

import concourse.bacc as bacc
import concourse.tile as tile
from concourse import mybir
from concourse.bass_utils import run_bass_kernel_spmd

P = 128
DIM = 1024
HEADS = 16
DHEAD = 64
INNER = HEADS * DHEAD
NCORES = 8
HPC = HEADS // NCORES  # heads per core (2)
CB = DIM // P  # contraction chunks (8)
IB = 512  # i-block (psum bank) width
ROPE_BASE = 10000.0
LN_EPS = 1e-5

F32 = mybir.dt.float32
BF16 = mybir.dt.bfloat16
AF = mybir.ActivationFunctionType
ALU = mybir.AluOpType

# global q/k row order: [h0 evens | h1 evens | h0 odds | h1 odds].
# _QROWS[r] = (head, dim) of global row r; partner(r) = r ^ 64.
_EVENS = np.arange(0, DHEAD, 2)
_ODDS = np.arange(1, DHEAD, 2)


def _qcols():
    """Column indices (within a head-pair's 128 cols) for the global order."""
    cols = np.concatenate(
        [
            0 * DHEAD + _EVENS,
            1 * DHEAD + _EVENS,
            0 * DHEAD + _ODDS,
            1 * DHEAD + _ODDS,
        ]
    )
    return cols


def _head_rows(i):
    """Global q/k rows belonging to head i (i in 0,1)."""
    return np.concatenate([np.arange(32) + i * 32, np.arange(32) + 64 + i * 32])


def _build_nc(N, debug_taps=False):
    """Build the SPMD Bass program for sequence length N (tokens)."""
    NT = N // P  # token tiles
    NIB = N // IB  # i-blocks
    assert N % IB == 0

    nc = bacc.Bacc("TRN2", target_bir_lowering=False, debug=False)

    x_d = nc.dram_tensor("x", (N, DIM), F32, kind="ExternalInput")
    wq_d = nc.dram_tensor("wq", (CB, P, P), BF16, kind="ExternalInput")
    wk_d = nc.dram_tensor("wk", (CB, P, P), BF16, kind="ExternalInput")
    wv_d = nc.dram_tensor("wv", (CB, P, P), BF16, kind="ExternalInput")
    wb_d = nc.dram_tensor("wb", (HPC, P, P), BF16, kind="ExternalInput")
    wo_d = nc.dram_tensor("wo", (P, DIM), BF16, kind="ExternalInput")
    cos_d = nc.dram_tensor("cosT", (P, N), F32, kind="ExternalInput")
    sin_d = nc.dram_tensor("sinT", (P, N), F32, kind="ExternalInput")
    id_d = nc.dram_tensor("ident", (P, P), BF16, kind="ExternalInput")
    out_d = nc.dram_tensor("out", (N, DIM), F32, kind="ExternalOutput")
    if debug_taps:
        dbg = {
            "dbg_xnT": nc.dram_tensor("dbg_xnT", (P, CB, N), BF16, kind="ExternalOutput"),
            "dbg_q": nc.dram_tensor("dbg_q", (P, N), BF16, kind="ExternalOutput"),
            "dbg_k": nc.dram_tensor("dbg_k", (P, N), BF16, kind="ExternalOutput"),
            "dbg_kt": nc.dram_tensor("dbg_kt", (HPC, P, N), BF16, kind="ExternalOutput"),
            "dbg_e0": nc.dram_tensor("dbg_e0", (HPC, P, N), BF16, kind="ExternalOutput"),
            "dbg_r": nc.dram_tensor("dbg_r", (HPC, 1, N), F32, kind="ExternalOutput"),
            "dbg_osc": nc.dram_tensor("dbg_osc", (P, N), BF16, kind="ExternalOutput"),
            "dbg_v": nc.dram_tensor("dbg_v", (P, NT, 2 * (DHEAD + 1)), BF16, kind="ExternalOutput"),
        }

    VW = DHEAD + 1

    with tile.TileContext(nc) as tc, ExitStack() as ctx:
        const = ctx.enter_context(tc.tile_pool(name="const", bufs=1))
        big = ctx.enter_context(tc.tile_pool(name="big", bufs=1))

        wq_sb = const.tile([P, CB, P], BF16)
        wk_sb = const.tile([P, CB, P], BF16)
        wv_sb = const.tile([P, CB, P], BF16)
        wb_sb = const.tile([P, HPC, P], BF16)
        wo_sb = const.tile([P, DIM], BF16)
        cos_sb = const.tile([P, N], F32)
        sin_sb = const.tile([P, N], F32)
        id_sb = const.tile([P, P], BF16)
        eps_sb = const.tile([P, 1], F32)
        zero_sb = const.tile([P, 1], F32)
        nc.vector.memset(eps_sb[:], LN_EPS)
        nc.vector.memset(zero_sb[:], 0.0)
        nc.sync.dma_start(wq_sb[:], wq_d[:].rearrange("a p m -> p a m"))
        nc.sync.dma_start(wk_sb[:], wk_d[:].rearrange("a p m -> p a m"))
        nc.sync.dma_start(wv_sb[:], wv_d[:].rearrange("a p m -> p a m"))
        nc.sync.dma_start(wb_sb[:], wb_d[:].rearrange("a p m -> p a m"))
        nc.sync.dma_start(wo_sb[:], wo_d[:])
        nc.sync.dma_start(cos_sb[:], cos_d[:])
        nc.sync.dma_start(sin_sb[:], sin_d[:])
        nc.sync.dma_start(id_sb[:], id_d[:])

        # long-lived activations
        xnT = big.tile([P, CB, N], BF16)  # xn transposed, c on partitions
        q_rope = big.tile([P, N], BF16)
        k_rope = big.tile([P, N], BF16)
        ktT = big.tile([P, HPC, N], BF16)  # zero-padded per head
        v_sb = big.tile([P, NT, HPC * VW], BF16)  # [keys, tile, head|ones]
        outT_sc = big.tile([P, N], BF16)  # scaled attn out, d on partitions

        nc.gpsimd.memset(v_sb[:], 1.0)

        # ---- Phase A: load x, LayerNorm, transpose to xnT ----
        with ExitStack() as actx:
            xp = actx.enter_context(tc.tile_pool(name="xp", bufs=3))
            sp = actx.enter_context(tc.tile_pool(name="sp", bufs=4))
            xnp = actx.enter_context(tc.tile_pool(name="xnp", bufs=3))
            tp = actx.enter_context(tc.tile_pool(name="tp", bufs=1, space="PSUM"))

            n_group = 4  # token tiles per transpose-psum batch
            for tg in range(NT // n_group):
                ps_t = [
                    tp.tile([P, n_group * P], BF16, tag=f"t{cb}", name=f"ps_t{cb}")
                    for cb in range(CB)
                ]
                for ti in range(n_group):
                    t = tg * n_group + ti
                    xt = xp.tile([P, DIM], F32, tag="x")
                    nc.sync.dma_start(xt[:], x_d[t * P : (t + 1) * P, :])
                    st = sp.tile([P, 2, 6], F32, tag="st")
                    nc.vector.bn_stats(st[:, 0, :], xt[:, 0:512])
                    nc.vector.bn_stats(st[:, 1, :], xt[:, 512:1024])
                    mv = sp.tile([P, 2], F32, tag="mv")
                    nc.vector.bn_aggr(mv[:], st[:])
                    rstd = sp.tile([P, 1], F32, tag="rstd")
                    nc.scalar.activation(rstd[:], mv[:, 1:2], AF.Sqrt, bias=eps_sb[:])
                    nc.vector.reciprocal(rstd[:], rstd[:])
                    xn = xnp.tile([P, DIM], BF16, tag="xn")
                    nc.vector.tensor_scalar(
                        xn[:], xt[:], mv[:, 0:1], rstd[:], ALU.subtract, ALU.mult
                    )
                    for cb in range(CB):
                        nc.tensor.transpose(
                            ps_t[cb][:, ti * P : (ti + 1) * P],
                            xn[:, cb * P : (cb + 1) * P],
                            id_sb[:],
                        )
                for cb in range(CB):
                    dst = xnT[:, cb, tg * n_group * P : (tg + 1) * n_group * P]
                    nc.scalar.copy(dst, ps_t[cb][:])

        # ---- Phase A2: projections + rope + bilinear + v ----
        with ExitStack() as actx:
            qkps = actx.enter_context(tc.tile_pool(name="qkps", bufs=4, space="PSUM"))
            vps = actx.enter_context(tc.tile_pool(name="vps", bufs=2, space="PSUM"))
            rtmp = actx.enter_context(tc.tile_pool(name="rtmp", bufs=4))

            for w_sb, dst in ((wq_sb, q_rope), (wk_sb, k_rope)):
                for ib in range(NIB):
                    sl = slice(ib * IB, (ib + 1) * IB)
                    ps_q = qkps.tile([P, IB], F32, tag="qk", name="ps_q")
                    for cb in range(CB):
                        nc.tensor.matmul(
                            ps_q[:],
                            w_sb[:, cb, :],
                            xnT[:, cb, sl],
                            start=(cb == 0),
                            stop=(cb == CB - 1),
                        )
                    tcos = rtmp.tile([P, IB], F32, tag="tcos")
                    nc.vector.tensor_mul(tcos[:], ps_q[:], cos_sb[:, sl])
                    tsin = rtmp.tile([P, IB], F32, tag="tsin")
                    nc.vector.tensor_mul(
                        tsin[0:64, :], ps_q[64:128, :], sin_sb[0:64, sl]
                    )
                    nc.vector.tensor_mul(
                        tsin[64:128, :], ps_q[0:64, :], sin_sb[64:128, sl]
                    )
                    nc.vector.tensor_add(dst[:, sl], tcos[:], tsin[:])

            # bilinear: ktT_h = scatter(Wb_h)^T-contract, zero-padded per head
            for h in range(HPC):
                for ib in range(NIB):
                    sl = slice(ib * IB, (ib + 1) * IB)
                    ps_kt = qkps.tile([P, IB], F32, tag="qk", name="ps_kt")
                    nc.tensor.matmul(
                        ps_kt[:], wb_sb[:, h, :], k_rope[:, sl], start=True, stop=True
                    )
                    if ib % 2 == 0:
                        nc.vector.tensor_copy(ktT[:, h, sl], ps_kt[:])
                    else:
                        nc.scalar.copy(ktT[:, h, sl], ps_kt[:])
            if debug_taps:
                nc.sync.dma_start(dbg["dbg_k"][:], k_rope[:])

            # v in natural layout [keys, d] with ones columns for row-sums
            for t in range(NT):
                ps_v = vps.tile([P, P], F32, tag="v")
                for cb in range(CB):
                    nc.tensor.matmul(
                        ps_v[:],
                        xnT[:, cb, t * P : (t + 1) * P],
                        wv_sb[:, cb, :],
                        start=(cb == 0),
                        stop=(cb == CB - 1),
                    )
                nc.vector.tensor_copy(v_sb[:, t, 0:DHEAD], ps_v[:, 0:DHEAD])
                nc.scalar.copy(v_sb[:, t, VW : VW + DHEAD], ps_v[:, DHEAD : 2 * DHEAD])

        if debug_taps:
            nc.sync.dma_start(dbg["dbg_xnT"][:], xnT[:])
            nc.sync.dma_start(dbg["dbg_q"][:], q_rope[:])
            nc.sync.dma_start(dbg["dbg_kt"][:], ktT[:].rearrange("p a n -> a p n"))
            nc.sync.dma_start(dbg["dbg_v"][:], v_sb[:])

        # ---- Phase B: attention per head + interleaved output projection ----
        with ExitStack() as actx:
            sps = actx.enter_context(tc.tile_pool(name="sps", bufs=1, space="PSUM"))
            avps = actx.enter_context(tc.tile_pool(name="avps", bufs=1, space="PSUM"))
            ep = actx.enter_context(tc.tile_pool(name="ep", bufs=1))
            rp = actx.enter_context(tc.tile_pool(name="rp", bufs=2))
            op = actx.enter_context(tc.tile_pool(name="op", bufs=3))

            def wo_project(trange):
                """Output projection for token tiles in trange (needs outT_sc)."""
                for t in trange:
                    ps_o = sps.tile([P, DIM], F32, tag="sim", name="ps_o")
                    for cc in range(2):
                        nc.tensor.matmul(
                            ps_o[:, cc * IB : (cc + 1) * IB],
                            outT_sc[:, t * P : (t + 1) * P],
                            wo_sb[:, cc * IB : (cc + 1) * IB],
                            start=True,
                            stop=True,
                        )
                    o_sb = op.tile([P, DIM], F32, tag="osb")
                    nc.vector.tensor_copy(o_sb[:, 0:IB], ps_o[:, 0:IB])
                    nc.scalar.copy(o_sb[:, IB:DIM], ps_o[:, IB : 2 * IB])
                    nc.sync.dma_start(out_d[t * P : (t + 1) * P, :], o_sb[:])

            for h in range(HPC):
                expT = []
                for j in range(NT):
                    ps_s = sps.tile([P, N], F32, tag="sim", name="ps_s")
                    for ib in range(NIB):
                        sl = slice(ib * IB, (ib + 1) * IB)
                        nc.tensor.matmul(
                            ps_s[:, sl],
                            ktT[:, h, j * P : (j + 1) * P],
                            q_rope[:, sl],
                            start=True,
                            stop=True,
                        )
                    e_j = ep.tile([P, N], BF16, tag=f"e{j}")
                    nc.scalar.activation(e_j[:], ps_s[:], AF.Exp, bias=zero_sb[:])
                    if debug_taps and j == 0:
                        nc.sync.dma_start(dbg["dbg_e0"][h], e_j[:])
                    expT.append(e_j)
                ps_av = avps.tile([DHEAD + 1, N], F32, tag="av")
                # i-block groups so scaling/Wo can start on the first half
                NG = 2 if NIB >= 2 else 1
                for grp in range(NG):
                    gibs = range(grp * NIB // NG, (grp + 1) * NIB // NG)
                    gsl = slice(grp * N // NG, (grp + 1) * N // NG)
                    for j in range(NT):
                        for ib in gibs:
                            sl = slice(ib * IB, (ib + 1) * IB)
                            nc.tensor.matmul(
                                ps_av[:, sl],
                                v_sb[:, j, h * VW : (h + 1) * VW],
                                expT[j][:, sl],
                                start=(j == 0),
                                stop=(j == NT - 1),
                            )
                    rs_h = rp.tile([1, N // NG], F32, tag="rs")
                    nc.scalar.copy(rs_h[:], ps_av[DHEAD : DHEAD + 1, gsl])
                    r_h = rp.tile([1, N // NG], F32, tag="r")
                    nc.vector.reciprocal_approx_fast(r_h[:], rs_h[:])
                    if debug_taps:
                        nc.sync.dma_start(dbg["dbg_r"][h, :, gsl], r_h[:])
                    rb_h = rp.tile([P, N // NG], F32, tag="rb")
                    nc.gpsimd.partition_broadcast(rb_h[:], r_h[:])
                    nc.vector.tensor_mul(
                        outT_sc[h * DHEAD : (h + 1) * DHEAD, gsl],
                        ps_av[0:DHEAD, gsl],
                        rb_h[h * DHEAD : (h + 1) * DHEAD, :],
                    )
                    if h == HPC - 1:
                        if debug_taps and grp == NG - 1:
                            nc.sync.dma_start(dbg["dbg_osc"][:], outT_sc[:])
                        tpg = NT // NG
                        wo_project(range(grp * tpg, (grp + 1) * tpg))

    nc.compile()
    return nc


def _rope_tables(N):
    theta = 1.0 / (ROPE_BASE ** (np.arange(0, DHEAD, 2, dtype=np.float64) / DHEAD))
    pos = np.arange(N, dtype=np.float64)
    freqs = pos[:, None] * theta[None, :]  # [N, 32]
    emb = np.concatenate([freqs, freqs], axis=-1)  # [N, 64]
    cos, sin = np.cos(emb), np.sin(emb)  # [N, 64]
    # tables in the global row order [h0e | h1e | h0o | h1o]:
    # row r (even block): coefficient of dim 2r'; odd block: dim 2r'+1.
    cosT = np.empty((P, N))
    sinT = np.empty((P, N))
    for hb in range(2):  # which head's 32-block within each half
        for r in range(32):
            cosT[hb * 32 + r] = cos[:, 2 * r]
            cosT[64 + hb * 32 + r] = cos[:, 2 * r + 1]
            # out_even = q_even*cos - q_odd*sin ; out_odd = q_odd*cos + q_even*sin
            sinT[hb * 32 + r] = -sin[:, 2 * r]
            sinT[64 + hb * 32 + r] = sin[:, 2 * r + 1]
    return (
        np.ascontiguousarray(cosT.astype(np.float32)),
        np.ascontiguousarray(sinT.astype(np.float32)),
    )


def _prep_inputs(x, gamma, Wq, Wkv, W_bilinear, Wo):
    """Slice/permute weights per core; returns list of 8 input dicts."""
    b, N, _ = x.shape
    x2d = np.ascontiguousarray(x.reshape(N, DIM)).astype(np.float32)
    cosT, sinT = _rope_tables(N)
    ident = np.eye(P, dtype=ml_dtypes.bfloat16)

    g = gamma.astype(np.float64)
    Wqg = g[:, None] * Wq.astype(np.float64) * (DHEAD**-0.5)
    Wkg = g[:, None] * Wkv[:, :INNER].astype(np.float64)
    Wvg = g[:, None] * Wkv[:, INNER:].astype(np.float64)

    qcols = _qcols()
    in_maps = []
    for c in range(NCORES):
        heads = [HPC * c + i for i in range(HPC)]
        # columns of the head-pair in global row order
        pair_cols = np.concatenate([np.arange(h * DHEAD, (h + 1) * DHEAD) for h in heads])
        gq = pair_cols[qcols]  # global row r <- original inner column gq[r]
        vcols = pair_cols
        wq_c = Wqg[:, gq].astype(ml_dtypes.bfloat16).reshape(CB, P, P)
        wk_c = Wkg[:, gq].astype(ml_dtypes.bfloat16).reshape(CB, P, P)
        wv_c = Wvg[:, vcols].astype(ml_dtypes.bfloat16).reshape(CB, P, P)
        # scattered block-diagonal bilinear weights, zero-padded per head:
        # row r (k_rope row, dim dk), col e (ktT row, dim de) nonzero only for
        # rows/cols of head h: wb[h][r, e] = W_bilinear[head][dk, de]
        wb_c = np.zeros((HPC, P, P), dtype=np.float64)
        dim_of_row = np.empty(P, dtype=np.int64)
        head_of_row = np.empty(P, dtype=np.int64)
        for i in range(HPC):
            rows = _head_rows(i)
            dims = np.concatenate([_EVENS, _ODDS])
            dim_of_row[rows] = dims
            head_of_row[rows] = i
        for i, h in enumerate(heads):
            rows = _head_rows(i)
            wb_h = W_bilinear[h].astype(np.float64)
            sub = wb_h[np.ix_(dim_of_row[rows], dim_of_row[rows])]
            wb_c[i][np.ix_(rows, rows)] = sub
        wo_c = Wo[vcols, :].astype(ml_dtypes.bfloat16)
        in_maps.append(
            {
                "x": x2d,
                "wq": np.ascontiguousarray(wq_c),
                "wk": np.ascontiguousarray(wk_c),
                "wv": np.ascontiguousarray(wv_c),
                "wb": np.ascontiguousarray(wb_c.astype(ml_dtypes.bfloat16)),
                "wo": np.ascontiguousarray(wo_c),
                "cosT": cosT,
                "sinT": sinT,
                "ident": ident,
            }
        )
    return in_maps


_NC_CACHE = {}


def _get_nc(N):
    if N not in _NC_CACHE:
        _NC_CACHE[N] = _build_nc(N)
    return _NC_CACHE[N]


def kernel(x, gamma, Wq, Wkv, W_bilinear, Wo, _trace=False, _trace_kwargs=None):
    b, N, dim = x.shape
    assert b == 1 and dim == DIM
    nc = _get_nc(N)
    in_maps = _prep_inputs(x, gamma, Wq, Wkv, W_bilinear, Wo)
    kw = {}
    if _trace:
        kw = {"trace": True, **(_trace_kwargs or {})}
    res = run_bass_kernel_spmd(nc, in_maps, core_ids=list(range(NCORES)), **kw)
    acc = np.zeros((N, DIM), dtype=np.float64)
    for c in range(NCORES):
        acc += res.results[c]["out"].astype(np.float64)
    out = acc.astype(np.float32).reshape(1, N, DIM)
    if _trace:
        return out, res
    return out
